# revision 18
# baseline (speedup 1.0000x reference)
"""Trainium2 Bass kernel for nn_DC_CRD_85779086836063 (gnn_message_passing).

Reference math (B,C,H,W = 32,64,128,128):
    wvec = mean(x, (2,3))                          # [B, C]
    diff = wvec[:,:,None] - wvec[:,None,:]         # [B, C, C]
    e = exp(-diff); T = |1 - e/(1+e)| - 1          # = sigmoid(diff) - 1
    A = 0.5*(T + T^T) * theta                      # sigmoid(d)+sigmoid(-d) = 1
                                                   # => T + T^T = -1 (exactly)
                                                   # => A = -0.5 * theta  (data-independent)
    H = relu(A @ x_flat)                           # [B, C, HW]
    out = (W_lin @ H)^T + b_lin  reshaped raw [HW,C] -> [C,H,W]

So per batch: out[b] (as [HW, C]) = (W_lin @ relu(-0.5 theta @ x[b]))^T + b_lin.

Sharding: pure data parallel, batch dim 32 -> 4 per core across 8 cores;
theta/W_lin/b_lin replicated.

Per-core dataflow (2-batch packing to fill 128 partitions, C=64), v6:
    Ablk = blockdiag(-0.5 theta^T, -0.5 theta^T)   [128,128] bf16 (lhsT of mm1)
    Wblk = blockdiag(W_lin^T, W_lin^T)             [128,128] bf16 (rhs of mm2)
    Constants are built without touching PE/PSUM: DVE 32x32 block transposes
    of theta/W_lin, then quad copies into the blockdiag tiles. bias_tile
    [128, 1024] (b_lin broadcast) is built via a 1-row PE matmul + doubling.
    Loads alternate between SWDGE cast-loads (f32 DRAM -> bf16 SBUF in the
    DMA datapath; a single SWDGE queue paces ~3.2us/MiB) and scalar-HWDGE
    f32 loads (full ring speed; cast split across ACT/DVE) so the two load
    streams run in parallel. The output is stored as bf16 (the correctness gate is rel err
    < 2e-2; the bf16 chain sits at ~5e-3), halving store HBM+fabric bytes;
    kernel() widens to f32 on the host during the unshard. Stores: sync
    HWDGE ring, dtype-matched bf16, draining at full ring rate. One-chunk software pipeline: phase 1 of chunk g
    (mm1 + relu) is emitted before phase 2 of chunk g-1 (mm2' + bias + store)
    so ACT's relu stream never waits behind a whole mm2' group on PE.
    per 1 MiB chunk (2048 cols, one load covering both packed batches):
      per 1024-col sub:
        ps1 = Ablk.T @ x2                  (PE, bf16, 2 512-col matmuls)
        h   = relu(ps1) -> bf16, permuted scatter (ACT)
      per 1024-col group (8 128-col blocks t):
        ps2[:, t] = h_block.T @ Wblk       (PE, bf16; out is [n, (bi,c)] --
                                            mm2 and the transpose fused into
                                            one data-as-lhsT matmul)
        o[:, :, 8s:8s+8, :] = ps2 + bias   (DVE tensor_add vs broadcast tile,
                                            the only PSUM->SBUF pass)
      one 0.5 MiB bf16 HWDGE store covering both batches -> out[b0/b1,n,c]

Numerics: theta, W, x, h rounded to bf16 => ~5e-3 rel err (gate is 2e-2).
"""

import os
import sys

sys.path.insert(0, "/opt/trn_rl_repo")

import numpy as np

import concourse.bacc as bacc
import concourse.mybir as mybir
from concourse import tile
from concourse.bass_utils import run_bass_kernel_spmd

dt = mybir.dt
AF = mybir.ActivationFunctionType

B, C, H, W = 32, 64, 128, 128
HW = H * W
NCORES = 8
BL = B // NCORES  # batches per core
PAIRS = BL // 2

DMACHUNK = 2048  # cols per chunk (1 MiB load, 1 MiB store)
SUB = 1024  # cols per phase tile (2 PSUM banks); matmuls are 512/128-col

VARIANT = os.environ.get("BASS_VARIANT", "v2")


def _build(variant: str):
    nc = bacc.Bacc("TRN2", target_bir_lowering=False, debug=False)

    x_d = nc.dram_tensor("x", [BL, C, HW], dt.float32, kind="ExternalInput")
    th_d = nc.dram_tensor("theta", [C, C], dt.float32, kind="ExternalInput")
    wl_d = nc.dram_tensor("W_lin", [C, C], dt.float32, kind="ExternalInput")
    bl_d = nc.dram_tensor("b_lin", [C], dt.float32, kind="ExternalInput")
    out_d = nc.dram_tensor("out", [BL, HW, C], dt.bfloat16, kind="ExternalOutput")

    R = DMACHUNK // 128  # output rows per partition per chunk (16)
    WW = SUB // R  # w-window per sub in the relu scatter (32)
    NSUB = DMACHUNK // SUB  # subs per chunk (4)
    NCHUNK = HW // DMACHUNK  # chunks per pair (8)

    with tile.TileContext(nc) as tc:
        with (
            tc.tile_pool(name="const", bufs=1) as const,
            tc.tile_pool(name="xfp", bufs=6) as xfp,
            tc.tile_pool(name="xp", bufs=8) as xp,
            tc.tile_pool(name="hp", bufs=3) as hp,
            tc.tile_pool(name="op", bufs=8) as op_,
        ):
            # ---------- first x loads: issue before anything else ----------
            # sync ring takes even chunks, scalar ring odd chunks. Tiles for
            # the whole kernel come from xp (bufs=4), so the first 4 loads can
            # be in flight while the constants pipeline below resolves.
            xsrc = x_d[:].rearrange("b c n -> (b c) n")
            xf_tiles = {}
            x_tiles = {}

            def load_chunk(g):
                pair, ci = divmod(g, NCHUNK)
                b0, n0 = 2 * pair, ci * DMACHUNK
                s = xsrc[b0 * C : (b0 + 2) * C, n0 : n0 + DMACHUNK]
                if g >= 2 and g % 2 == 0:
                    # SWDGE cast-load: bf16 lands directly (half the SBUF
                    # fabric bytes, no compute cost) -- but a single SWDGE
                    # queue paces ~3.2 us per 1 MiB, so odd chunks take the
                    # parallel HWDGE path below
                    x2 = xp.tile([128, DMACHUNK], dt.bfloat16, tag="x2")
                    nc.gpsimd.dma_start(x2[:], s)
                    x_tiles[g] = x2
                else:
                    # scalar-HWDGE f32 load at full ring speed (also the head
                    # chunks: the SWDGE path sits behind the engine preamble);
                    # bf16 cast split across ACT/DVE afterwards
                    xf = xfp.tile([128, DMACHUNK], dt.float32, tag="xf")
                    nc.scalar.dma_start(xf[:], s)
                    xf_tiles[g] = xf

            def cast_chunk(g):
                if g not in xf_tiles:
                    return
                xf = xf_tiles.pop(g)
                x2 = xp.tile([128, DMACHUNK], dt.bfloat16, tag="x2")
                half = DMACHUNK // 2
                nc.scalar.activation(x2[:, 0:half], xf[:, 0:half], AF.Copy)
                nc.vector.tensor_copy(x2[:, half:DMACHUNK], xf[:, half:DMACHUNK])
                x_tiles[g] = x2

            load_chunk(0)

            # ---------------- constants (no PE transposes, no identity) ----
            # blockdiag targets zeroed on gpsimd; theta/W/bias DMAs on the
            # sync HWDGE ring (tiny; stores only start much later); DVE does
            # the 32x32 block transposes and the quad copies.
            ablk = const.tile([128, 128], dt.bfloat16, tag="ablk")
            wblk = const.tile([128, 128], dt.bfloat16, tag="wblk")
            ones_r = const.tile([1, 128], dt.float32, tag="ones_r")
            nc.gpsimd.memset(ablk[:], 0.0)
            nc.gpsimd.memset(wblk[:], 0.0)
            nc.gpsimd.memset(ones_r[:], 1.0)

            th_t = const.tile([64, 64], dt.float32, tag="th_t")
            wl_t = const.tile([64, 64], dt.float32, tag="wl_t")
            bias_r = const.tile([1, 64], dt.float32, tag="bias_r")
            nc.sync.dma_start(th_t[:], th_d[:])
            nc.sync.dma_start(wl_t[:], wl_d[:])
            nc.sync.dma_start(bias_r[:], bl_d[:].rearrange("(one c) -> one c", one=1))
            for _g in range(1, 6):
                load_chunk(_g)
            cast_chunk(0)
            cast_chunk(1)

            # full 64x64 transposes out of 32x32 DVE block transposes
            thT = const.tile([64, 64], dt.float32, tag="thT")
            wlT = const.tile([64, 64], dt.float32, tag="wlT")
            for src_t, dst_t in ((th_t, thT), (wl_t, wlT)):
                for i in (0, 32):
                    for j in (0, 32):
                        nc.vector.transpose(
                            dst_t[i : i + 32, j : j + 32],
                            src_t[j : j + 32, i : i + 32],
                        )
            # Ablk = -0.5 * blockdiag(theta^T, theta^T)  (lhsT of mm1, bf16)
            nc.vector.tensor_scalar_mul(ablk[0:64, 0:64], thT[:], -0.5)
            nc.vector.tensor_scalar_mul(ablk[64:128, 64:128], thT[:], -0.5)
            # Wblk = blockdiag(W_lin^T, W_lin^T)  (rhs of mm2, bf16)
            nc.vector.tensor_copy(wblk[0:64, 0:64], wlT[:])
            nc.vector.tensor_copy(wblk[64:128, 64:128], wlT[:])

            # bias_tile [128, (t4 bi c)]: b_lin broadcast to all partitions
            # via a K=1 PE matmul, then log-doubled across the free dim
            psc_cm = tc.tile_pool(name="psc", bufs=1, space="PSUM")
            psc = psc_cm.__enter__()
            psb = psc.tile([128, 64], dt.float32, tag="psb")
            nc.tensor.matmul(psb[:], ones_r[:], bias_r[:], start=True, stop=True)
            bias_t = const.tile([128, 1024], dt.float32, tag="bias_t")
            nc.vector.tensor_copy(bias_t[:, 0:64], psb[:])
            nc.vector.tensor_copy(bias_t[:, 64:128], bias_t[:, 0:64])
            nc.vector.tensor_copy(bias_t[:, 128:256], bias_t[:, 0:128])
            nc.vector.tensor_copy(bias_t[:, 256:512], bias_t[:, 0:256])
            nc.vector.tensor_copy(bias_t[:, 512:1024], bias_t[:, 0:512])
            bias_v = bias_t[:].rearrange("p (bi t c) -> p bi t c", bi=2, c=64)

            psc_cm.__exit__(None, None, None)
            ps1p_cm = tc.tile_pool(name="ps1p", bufs=2, space="PSUM")
            ps2p_cm = tc.tile_pool(name="ps2p", bufs=2, space="PSUM")
            ps1p = ps1p_cm.__enter__()
            ps2p = ps2p_cm.__enter__()

            # ---------------- main loop (one-chunk software pipeline) ----
            # Phase 1 of chunk g (mm1 + relu) is emitted BEFORE phase 2 of
            # chunk g-1 (mm2' + bias-add + store), so the PE queue runs
            # mm1(g) ahead of mm2'(g-1) and ACT's relu stream never waits a
            # whole mm2' group.
            loaded = set(range(6))

            def phase1(g):
                if g + 6 < PAIRS * NCHUNK and (g + 6) not in loaded:
                    loaded.add(g + 6)
                    load_chunk(g + 6)
                x2 = x_tiles.pop(g)
                h = hp.tile([128, DMACHUNK], dt.bfloat16, tag="h")
                hv = h[:].rearrange("p (t q) -> p t q", t=R)
                for s in range(NSUB):
                    ps1 = ps1p.tile([128, SUB], dt.float32, tag="ps1")
                    for m in range(SUB // 512):
                        c0 = s * SUB + m * 512
                        nc.tensor.matmul(
                            ps1[:, m * 512 : (m + 1) * 512],
                            ablk[:],
                            x2[:, c0 : c0 + 512],
                            start=True,
                            stop=True,
                        )
                    # relu + permuted scatter over the whole 1024-col tile
                    ps1v = ps1[:].rearrange("p (a r) -> p r a", r=R)
                    nc.scalar.activation(
                        hv[:, :, WW * s : WW * (s + 1)], ps1v, AF.Relu
                    )
                # cast the chunk-after-next now, between this chunk's relus
                # and the next chunk's, so ACT never stalls ahead of a relu
                if g + 2 < PAIRS * NCHUNK:
                    cast_chunk(g + 2)
                return h

            def phase2(g, h):
                pair, ci = divmod(g, NCHUNK)
                b0, n0 = 2 * pair, ci * DMACHUNK
                o = op_.tile([128, DMACHUNK], dt.bfloat16, tag="o")
                obv = o[:].rearrange("p (bi tc c) -> p bi tc c", bi=2, c=64)
                TG = SUB // 128  # mm2' blocks per group (8)
                for s in range(NSUB):
                    ps2 = ps2p.tile([128, SUB], dt.float32, tag="ps2")
                    for t in range(TG):
                        blk = (TG * s + t) * 128
                        nc.tensor.matmul(
                            ps2[:, t * 128 : (t + 1) * 128],
                            h[:, blk : blk + 128],
                            wblk[:],
                            start=True,
                            stop=True,
                        )
                    p2v = ps2[:].rearrange("p (t bi c) -> p bi t c", t=TG, c=64)
                    dst = obv[:, :, TG * s : TG * (s + 1), :]
                    nc.vector.tensor_add(dst, p2v, bias_v)
                dd = out_d[b0 : b0 + 2, n0 : n0 + DMACHUNK, :].rearrange(
                    "bi (p tc) c -> p bi tc c", p=128
                )
                nc.sync.dma_start(dd, obv)

            prev = None
            for g in range(PAIRS * NCHUNK):
                h = phase1(g)
                if prev is not None:
                    phase2(prev[0], prev[1])
                prev = (g, h)
            phase2(prev[0], prev[1])
            ps2p_cm.__exit__(None, None, None)
            ps1p_cm.__exit__(None, None, None)

    nc.compile()
    return nc


def _ensure_ntff_hook():
    """Register the axon NTFF profile hook (profiling only; best-effort).

    The agent image's ``antenv`` lacks ``axon_hooks``, so ``trace=True`` in
    ``run_bass_kernel_spmd`` would ImportError. Recreate the module with the
    same ctypes hook ``trn_agent_boot.trn_boot`` would have registered.
    """
    import contextlib
    import ctypes
    import types

    if "antenv.axon_hooks" in sys.modules:
        return
    so_path = "/opt/axon/libaxon_pjrt.so"
    try:
        lib = ctypes.CDLL(so_path)
        lib.axon_start_nrt_profile.argtypes = [
            ctypes.POINTER(ctypes.c_int64),
            ctypes.c_size_t,
        ]
        lib.axon_start_nrt_profile.restype = ctypes.c_int64
        lib.axon_stop_nrt_profile.argtypes = [ctypes.c_char_p]
        lib.axon_stop_nrt_profile.restype = ctypes.c_int64
    except (OSError, AttributeError):
        lib = None

    @contextlib.contextmanager
    def _hook(output_dir, device_ids):
        import jax

        jax.devices()
        if device_ids:
            ids = (ctypes.c_int64 * len(device_ids))(*device_ids)
            rc = lib.axon_start_nrt_profile(ids, len(device_ids))
        else:
            rc = lib.axon_start_nrt_profile(None, 0)
        if rc != 0:
            raise RuntimeError(f"axon_start_nrt_profile rc={rc}")
        try:
            yield
        finally:
            n = lib.axon_stop_nrt_profile(str(output_dir).encode())
            print(f"ntff profile: {n} file(s) written to {output_dir}")

    hook = _hook if lib is not None else None
    mod = types.ModuleType("antenv.axon_hooks")
    mod.get_axon_ntff_profile_hook = lambda: hook
    mod.set_axon_ntff_profile_hook = lambda h: None
    sys.modules["antenv.axon_hooks"] = mod


_NC_CACHE = {}


def _get_nc(variant: str):
    if variant not in _NC_CACHE:
        _NC_CACHE[variant] = _build(variant)
    return _NC_CACHE[variant]


def _run(inputs: dict, trace: bool = False, variant: str | None = None):
    variant = variant or VARIANT
    if trace:
        _ensure_ntff_hook()
    nc = _get_nc(variant)
    x = np.ascontiguousarray(inputs["x"], dtype=np.float32)
    theta = np.ascontiguousarray(inputs["theta"], dtype=np.float32)
    w_lin = np.ascontiguousarray(inputs["W_lin"], dtype=np.float32)
    b_lin = np.ascontiguousarray(inputs["b_lin"], dtype=np.float32)
    in_maps = [
        {
            "x": np.ascontiguousarray(x[i * BL : (i + 1) * BL].reshape(BL, C, HW)),
            "theta": theta,
            "W_lin": w_lin,
            "b_lin": b_lin,
        }
        for i in range(NCORES)
    ]
    # Occasionally the first execution of a freshly-loaded NEFF fails with
    # NRT_EXEC_UNIT_UNRECOVERABLE; a retry on the recovered device succeeds.
    import time

    last_err = None
    for attempt in range(4):
        try:
            res = run_bass_kernel_spmd(
                nc,
                in_maps,
                core_ids=list(range(NCORES)),
                trace=trace and attempt == 0,
            )
            break
        except Exception as e:  # noqa: BLE001
            last_err = e
            try:  # drop the (possibly dead) PJRT client; next call re-inits
                import jax

                jax.clear_caches()
                jax.extend.backend.clear_backends()
            except Exception:  # noqa: BLE001
                pass
            time.sleep(10 * (attempt + 1))
    else:
        raise last_err
    shards = [
        np.asarray(r["out"]).astype(np.float32).reshape(BL, C, H, W)
        for r in res.results
    ]
    return np.concatenate(shards, axis=0), res


def kernel(x, theta, W_lin, b_lin):
    out, _ = _run({"x": x, "theta": theta, "W_lin": W_lin, "b_lin": b_lin})
    return out


# revision 19
# speedup vs baseline: 1.0452x; 1.0452x over previous
"""Trainium2 Bass kernel for nn_DC_CRD_85779086836063 (gnn_message_passing).

Reference math (B,C,H,W = 32,64,128,128):
    wvec = mean(x, (2,3))                          # [B, C]
    diff = wvec[:,:,None] - wvec[:,None,:]         # [B, C, C]
    e = exp(-diff); T = |1 - e/(1+e)| - 1          # = sigmoid(diff) - 1
    A = 0.5*(T + T^T) * theta                      # sigmoid(d)+sigmoid(-d) = 1
                                                   # => T + T^T = -1 (exactly)
                                                   # => A = -0.5 * theta  (data-independent)
    H = relu(A @ x_flat)                           # [B, C, HW]
    out = (W_lin @ H)^T + b_lin  reshaped raw [HW,C] -> [C,H,W]

So per batch: out[b] (as [HW, C]) = (W_lin @ relu(-0.5 theta @ x[b]))^T + b_lin.

Sharding: pure data parallel, batch dim 32 -> 4 per core across 8 cores;
theta/W_lin/b_lin replicated.

Per-core dataflow (2-batch packing to fill 128 partitions, C=64), v6:
    Ablk = blockdiag(-0.5 theta^T, -0.5 theta^T)   [128,128] bf16 (lhsT of mm1)
    Wblk = blockdiag(W_lin^T, W_lin^T)             [128,128] bf16 (rhs of mm2)
    Constants are built without touching PE/PSUM: DVE 32x32 block transposes
    of theta/W_lin, then quad copies into the blockdiag tiles. bias_tile
    [128, 1024] (b_lin broadcast) is built via a 1-row PE matmul + doubling.
    Loads are 2 MiB SWDGE cast-loads (f32 DRAM -> bf16 SBUF in the DMA
    datapath; bigger chunks amortize the SWDGE per-DMA pacing overhead);
    chunk 0 rides scalar-HWDGE f32 + ACT/DVE cast. The output is stored as bf16 (the correctness gate is rel err
    < 2e-2; the bf16 chain sits at ~5e-3), halving store HBM+fabric bytes;
    kernel() widens to f32 on the host during the unshard. Stores: sync
    HWDGE ring, dtype-matched bf16, draining at full ring rate. One-chunk software pipeline: phase 1 of chunk g
    (mm1 + relu) is emitted before phase 2 of chunk g-1 (mm2' + bias + store)
    so ACT's relu stream never waits behind a whole mm2' group on PE.
    per 1 MiB chunk (2048 cols, one load covering both packed batches):
      per 1024-col sub:
        ps1 = Ablk.T @ x2                  (PE, bf16, 2 512-col matmuls)
        h   = relu(ps1) -> bf16, permuted scatter (ACT)
      per 1024-col group (8 128-col blocks t):
        ps2[:, t] = h_block.T @ Wblk       (PE, bf16; out is [n, (bi,c)] --
                                            mm2 and the transpose fused into
                                            one data-as-lhsT matmul)
        o[:, :, 8s:8s+8, :] = ps2 + bias   (DVE tensor_add vs broadcast tile,
                                            the only PSUM->SBUF pass)
      one 0.5 MiB bf16 HWDGE store covering both batches -> out[b0/b1,n,c]

Numerics: theta, W, x, h rounded to bf16 => ~5e-3 rel err (gate is 2e-2).
"""

import os
import sys

sys.path.insert(0, "/opt/trn_rl_repo")

import numpy as np

import concourse.bacc as bacc
import concourse.mybir as mybir
from concourse import tile
from concourse.bass_utils import run_bass_kernel_spmd

dt = mybir.dt
AF = mybir.ActivationFunctionType

B, C, H, W = 32, 64, 128, 128
HW = H * W
NCORES = 8
BL = B // NCORES  # batches per core
PAIRS = BL // 2

DMACHUNK = 4096  # cols per chunk (2 MiB f32 load, 1 MiB bf16 store)
SUB = 1024  # cols per phase tile (2 PSUM banks); matmuls are 512/128-col

VARIANT = os.environ.get("BASS_VARIANT", "v2")


def _build(variant: str):
    nc = bacc.Bacc("TRN2", target_bir_lowering=False, debug=False)

    x_d = nc.dram_tensor("x", [BL, C, HW], dt.float32, kind="ExternalInput")
    th_d = nc.dram_tensor("theta", [C, C], dt.float32, kind="ExternalInput")
    wl_d = nc.dram_tensor("W_lin", [C, C], dt.float32, kind="ExternalInput")
    bl_d = nc.dram_tensor("b_lin", [C], dt.float32, kind="ExternalInput")
    out_d = nc.dram_tensor("out", [BL, HW, C], dt.bfloat16, kind="ExternalOutput")

    R = DMACHUNK // 128  # output rows per partition per chunk (16)
    WW = SUB // R  # w-window per sub in the relu scatter (32)
    NSUB = DMACHUNK // SUB  # subs per chunk (4)
    NCHUNK = HW // DMACHUNK  # chunks per pair (8)

    with tile.TileContext(nc) as tc:
        with (
            tc.tile_pool(name="const", bufs=1) as const,
            tc.tile_pool(name="xfp", bufs=1) as xfp,
            tc.tile_pool(name="xp", bufs=4) as xp,
            tc.tile_pool(name="hp", bufs=3) as hp,
            tc.tile_pool(name="op", bufs=4) as op_,
        ):
            # ---------- first x loads: issue before anything else ----------
            # sync ring takes even chunks, scalar ring odd chunks. Tiles for
            # the whole kernel come from xp (bufs=4), so the first 4 loads can
            # be in flight while the constants pipeline below resolves.
            xsrc = x_d[:].rearrange("b c n -> (b c) n")
            xf_tiles = {}
            x_tiles = {}

            def load_chunk(g):
                pair, ci = divmod(g, NCHUNK)
                b0, n0 = 2 * pair, ci * DMACHUNK
                s = xsrc[b0 * C : (b0 + 2) * C, n0 : n0 + DMACHUNK]
                if g >= 1:
                    # SWDGE cast-load: bf16 lands directly -- half the SBUF
                    # fabric bytes, no compute cost
                    x2 = xp.tile([128, DMACHUNK], dt.bfloat16, tag="x2")
                    nc.gpsimd.dma_start(x2[:], s)
                    x_tiles[g] = x2
                else:
                    # chunk 0 rides scalar-HWDGE f32 (the SWDGE path sits
                    # behind the engine preamble); bf16 cast split ACT/DVE
                    xf = xfp.tile([128, DMACHUNK], dt.float32, tag="xf")
                    nc.scalar.dma_start(xf[:], s)
                    xf_tiles[g] = xf

            def cast_chunk(g):
                if g not in xf_tiles:
                    return
                xf = xf_tiles.pop(g)
                x2 = xp.tile([128, DMACHUNK], dt.bfloat16, tag="x2")
                q = DMACHUNK // 4
                for i in range(4):
                    eng = nc.scalar if i % 2 == 0 else nc.vector
                    sl = slice(i * q, (i + 1) * q)
                    if i % 2 == 0:
                        nc.scalar.activation(x2[:, sl], xf[:, sl], AF.Copy)
                    else:
                        nc.vector.tensor_copy(x2[:, sl], xf[:, sl])
                x_tiles[g] = x2

            load_chunk(0)

            # ---------------- constants (no PE transposes, no identity) ----
            # blockdiag targets zeroed on gpsimd; theta/W/bias DMAs on the
            # sync HWDGE ring (tiny; stores only start much later); DVE does
            # the 32x32 block transposes and the quad copies.
            ablk = const.tile([128, 128], dt.bfloat16, tag="ablk")
            wblk = const.tile([128, 128], dt.bfloat16, tag="wblk")
            ones_r = const.tile([1, 128], dt.float32, tag="ones_r")
            load_chunk(1)
            nc.gpsimd.memset(ablk[:], 0.0)
            nc.gpsimd.memset(wblk[:], 0.0)
            nc.gpsimd.memset(ones_r[:], 1.0)

            th_t = const.tile([64, 64], dt.float32, tag="th_t")
            wl_t = const.tile([64, 64], dt.float32, tag="wl_t")
            bias_r = const.tile([1, 64], dt.float32, tag="bias_r")
            nc.sync.dma_start(th_t[:], th_d[:])
            nc.sync.dma_start(wl_t[:], wl_d[:])
            nc.sync.dma_start(bias_r[:], bl_d[:].rearrange("(one c) -> one c", one=1))
            load_chunk(2)
            cast_chunk(0)

            # full 64x64 transposes out of 32x32 DVE block transposes
            thT = const.tile([64, 64], dt.float32, tag="thT")
            wlT = const.tile([64, 64], dt.float32, tag="wlT")
            for src_t, dst_t in ((th_t, thT), (wl_t, wlT)):
                for i in (0, 32):
                    for j in (0, 32):
                        nc.vector.transpose(
                            dst_t[i : i + 32, j : j + 32],
                            src_t[j : j + 32, i : i + 32],
                        )
            # Ablk = -0.5 * blockdiag(theta^T, theta^T)  (lhsT of mm1, bf16)
            nc.vector.tensor_scalar_mul(ablk[0:64, 0:64], thT[:], -0.5)
            nc.vector.tensor_scalar_mul(ablk[64:128, 64:128], thT[:], -0.5)
            # Wblk = blockdiag(W_lin^T, W_lin^T)  (rhs of mm2, bf16)
            nc.vector.tensor_copy(wblk[0:64, 0:64], wlT[:])
            nc.vector.tensor_copy(wblk[64:128, 64:128], wlT[:])

            # bias_tile [128, (t4 bi c)]: b_lin broadcast to all partitions
            # via a K=1 PE matmul, then log-doubled across the free dim
            psc_cm = tc.tile_pool(name="psc", bufs=1, space="PSUM")
            psc = psc_cm.__enter__()
            psb = psc.tile([128, 64], dt.float32, tag="psb")
            nc.tensor.matmul(psb[:], ones_r[:], bias_r[:], start=True, stop=True)
            bias_t = const.tile([128, 1024], dt.float32, tag="bias_t")
            nc.vector.tensor_copy(bias_t[:, 0:64], psb[:])
            nc.vector.tensor_copy(bias_t[:, 64:128], bias_t[:, 0:64])
            nc.vector.tensor_copy(bias_t[:, 128:256], bias_t[:, 0:128])
            nc.vector.tensor_copy(bias_t[:, 256:512], bias_t[:, 0:256])
            nc.vector.tensor_copy(bias_t[:, 512:1024], bias_t[:, 0:512])
            bias_v = bias_t[:].rearrange("p (bi t c) -> p bi t c", bi=2, c=64)

            psc_cm.__exit__(None, None, None)
            ps1p_cm = tc.tile_pool(name="ps1p", bufs=2, space="PSUM")
            ps2p_cm = tc.tile_pool(name="ps2p", bufs=2, space="PSUM")
            ps1p = ps1p_cm.__enter__()
            ps2p = ps2p_cm.__enter__()

            # ---------------- main loop (one-chunk software pipeline) ----
            # Phase 1 of chunk g (mm1 + relu) is emitted BEFORE phase 2 of
            # chunk g-1 (mm2' + bias-add + store), so the PE queue runs
            # mm1(g) ahead of mm2'(g-1) and ACT's relu stream never waits a
            # whole mm2' group.
            loaded = set(range(3))

            def phase1(g):
                if g + 3 < PAIRS * NCHUNK and (g + 3) not in loaded:
                    loaded.add(g + 3)
                    load_chunk(g + 3)
                x2 = x_tiles.pop(g)
                h = hp.tile([128, DMACHUNK], dt.bfloat16, tag="h")
                hv = h[:].rearrange("p (t q) -> p t q", t=R)
                for s in range(NSUB):
                    ps1 = ps1p.tile([128, SUB], dt.float32, tag="ps1")
                    for m in range(SUB // 512):
                        c0 = s * SUB + m * 512
                        nc.tensor.matmul(
                            ps1[:, m * 512 : (m + 1) * 512],
                            ablk[:],
                            x2[:, c0 : c0 + 512],
                            start=True,
                            stop=True,
                        )
                    # relu + permuted scatter over the whole 1024-col tile
                    ps1v = ps1[:].rearrange("p (a r) -> p r a", r=R)
                    nc.scalar.activation(
                        hv[:, :, WW * s : WW * (s + 1)], ps1v, AF.Relu
                    )
                # cast the chunk-after-next now, between this chunk's relus
                # and the next chunk's, so ACT never stalls ahead of a relu
                if g + 2 < PAIRS * NCHUNK:
                    cast_chunk(g + 2)
                return h

            def phase2(g, h):
                pair, ci = divmod(g, NCHUNK)
                b0, n0 = 2 * pair, ci * DMACHUNK
                o = op_.tile([128, DMACHUNK], dt.bfloat16, tag="o")
                obv = o[:].rearrange("p (bi tc c) -> p bi tc c", bi=2, c=64)
                TG = SUB // 128  # mm2' blocks per group (8)
                for s in range(NSUB):
                    ps2 = ps2p.tile([128, SUB], dt.float32, tag="ps2")
                    for t in range(TG):
                        blk = (TG * s + t) * 128
                        nc.tensor.matmul(
                            ps2[:, t * 128 : (t + 1) * 128],
                            h[:, blk : blk + 128],
                            wblk[:],
                            start=True,
                            stop=True,
                        )
                    p2v = ps2[:].rearrange("p (t bi c) -> p bi t c", t=TG, c=64)
                    dst = obv[:, :, TG * s : TG * (s + 1), :]
                    nc.vector.tensor_add(dst, p2v, bias_v)
                dd = out_d[b0 : b0 + 2, n0 : n0 + DMACHUNK, :].rearrange(
                    "bi (p tc) c -> p bi tc c", p=128
                )
                nc.sync.dma_start(dd, obv)

            prev = None
            for g in range(PAIRS * NCHUNK):
                h = phase1(g)
                if prev is not None:
                    phase2(prev[0], prev[1])
                prev = (g, h)
            phase2(prev[0], prev[1])
            ps2p_cm.__exit__(None, None, None)
            ps1p_cm.__exit__(None, None, None)

    nc.compile()
    return nc


def _ensure_ntff_hook():
    """Register the axon NTFF profile hook (profiling only; best-effort).

    The agent image's ``antenv`` lacks ``axon_hooks``, so ``trace=True`` in
    ``run_bass_kernel_spmd`` would ImportError. Recreate the module with the
    same ctypes hook ``trn_agent_boot.trn_boot`` would have registered.
    """
    import contextlib
    import ctypes
    import types

    if "antenv.axon_hooks" in sys.modules:
        return
    so_path = "/opt/axon/libaxon_pjrt.so"
    try:
        lib = ctypes.CDLL(so_path)
        lib.axon_start_nrt_profile.argtypes = [
            ctypes.POINTER(ctypes.c_int64),
            ctypes.c_size_t,
        ]
        lib.axon_start_nrt_profile.restype = ctypes.c_int64
        lib.axon_stop_nrt_profile.argtypes = [ctypes.c_char_p]
        lib.axon_stop_nrt_profile.restype = ctypes.c_int64
    except (OSError, AttributeError):
        lib = None

    @contextlib.contextmanager
    def _hook(output_dir, device_ids):
        import jax

        jax.devices()
        if device_ids:
            ids = (ctypes.c_int64 * len(device_ids))(*device_ids)
            rc = lib.axon_start_nrt_profile(ids, len(device_ids))
        else:
            rc = lib.axon_start_nrt_profile(None, 0)
        if rc != 0:
            raise RuntimeError(f"axon_start_nrt_profile rc={rc}")
        try:
            yield
        finally:
            n = lib.axon_stop_nrt_profile(str(output_dir).encode())
            print(f"ntff profile: {n} file(s) written to {output_dir}")

    hook = _hook if lib is not None else None
    mod = types.ModuleType("antenv.axon_hooks")
    mod.get_axon_ntff_profile_hook = lambda: hook
    mod.set_axon_ntff_profile_hook = lambda h: None
    sys.modules["antenv.axon_hooks"] = mod


_NC_CACHE = {}


def _get_nc(variant: str):
    if variant not in _NC_CACHE:
        _NC_CACHE[variant] = _build(variant)
    return _NC_CACHE[variant]


def _run(inputs: dict, trace: bool = False, variant: str | None = None):
    variant = variant or VARIANT
    if trace:
        _ensure_ntff_hook()
    nc = _get_nc(variant)
    x = np.ascontiguousarray(inputs["x"], dtype=np.float32)
    theta = np.ascontiguousarray(inputs["theta"], dtype=np.float32)
    w_lin = np.ascontiguousarray(inputs["W_lin"], dtype=np.float32)
    b_lin = np.ascontiguousarray(inputs["b_lin"], dtype=np.float32)
    in_maps = [
        {
            "x": np.ascontiguousarray(x[i * BL : (i + 1) * BL].reshape(BL, C, HW)),
            "theta": theta,
            "W_lin": w_lin,
            "b_lin": b_lin,
        }
        for i in range(NCORES)
    ]
    # Occasionally the first execution of a freshly-loaded NEFF fails with
    # NRT_EXEC_UNIT_UNRECOVERABLE; a retry on the recovered device succeeds.
    import time

    last_err = None
    for attempt in range(4):
        try:
            res = run_bass_kernel_spmd(
                nc,
                in_maps,
                core_ids=list(range(NCORES)),
                trace=trace and attempt == 0,
            )
            break
        except Exception as e:  # noqa: BLE001
            last_err = e
            try:  # drop the (possibly dead) PJRT client; next call re-inits
                import jax

                jax.clear_caches()
                jax.extend.backend.clear_backends()
            except Exception:  # noqa: BLE001
                pass
            time.sleep(10 * (attempt + 1))
    else:
        raise last_err
    shards = [
        np.asarray(r["out"]).astype(np.float32).reshape(BL, C, H, W)
        for r in res.results
    ]
    return np.concatenate(shards, axis=0), res


def kernel(x, theta, W_lin, b_lin):
    out, _ = _run({"x": x, "theta": theta, "W_lin": W_lin, "b_lin": b_lin})
    return out


# revision 20
# speedup vs baseline: 1.0619x; 1.0160x over previous
"""Trainium2 Bass kernel for nn_DC_CRD_85779086836063 (gnn_message_passing).

Reference math (B,C,H,W = 32,64,128,128):
    wvec = mean(x, (2,3))                          # [B, C]
    diff = wvec[:,:,None] - wvec[:,None,:]         # [B, C, C]
    e = exp(-diff); T = |1 - e/(1+e)| - 1          # = sigmoid(diff) - 1
    A = 0.5*(T + T^T) * theta                      # sigmoid(d)+sigmoid(-d) = 1
                                                   # => T + T^T = -1 (exactly)
                                                   # => A = -0.5 * theta  (data-independent)
    H = relu(A @ x_flat)                           # [B, C, HW]
    out = (W_lin @ H)^T + b_lin  reshaped raw [HW,C] -> [C,H,W]

So per batch: out[b] (as [HW, C]) = (W_lin @ relu(-0.5 theta @ x[b]))^T + b_lin.

Sharding: pure data parallel, batch dim 32 -> 4 per core across 8 cores;
theta/W_lin/b_lin replicated.

Per-core dataflow (2-batch packing to fill 128 partitions, C=64), v6:
    Ablk = blockdiag(-0.5 theta^T, -0.5 theta^T)   [128,128] bf16 (lhsT of mm1)
    Wblk = blockdiag(W_lin^T, W_lin^T)             [128,128] bf16 (rhs of mm2)
    Constants are built without touching PE/PSUM: DVE 32x32 block transposes
    of theta/W_lin, then quad copies into the blockdiag tiles. bias_tile
    [128, 1024] (b_lin broadcast) is built via a 1-row PE matmul + doubling.
    Loads are 2 MiB SWDGE cast-loads (f32 DRAM -> bf16 SBUF in the DMA
    datapath; bigger chunks amortize the SWDGE per-DMA pacing overhead);
    chunk 0 rides scalar-HWDGE f32 + ACT/DVE cast. The output is stored as bf16 (the correctness gate is rel err
    < 2e-2; the bf16 chain sits at ~5e-3), halving store HBM+fabric bytes;
    kernel() widens to f32 on the host during the unshard. Stores: sync
    HWDGE ring, dtype-matched bf16, draining at full ring rate. One-chunk software pipeline: phase 1 of chunk g
    (mm1 + relu) is emitted before phase 2 of chunk g-1 (mm2' + bias + store)
    so ACT's relu stream never waits behind a whole mm2' group on PE.
    per 1 MiB chunk (2048 cols, one load covering both packed batches):
      per 1024-col sub:
        ps1 = Ablk.T @ x2                  (PE, bf16, 2 512-col matmuls)
        h   = relu(ps1) -> bf16, permuted scatter (ACT)
      per 1024-col group (8 128-col blocks t):
        ps2[:, t] = h_block.T @ Wblk       (PE, bf16; out is [n, (bi,c)] --
                                            mm2 and the transpose fused into
                                            one data-as-lhsT matmul)
        o[:, :, 8s:8s+8, :] = ps2 + bias   (DVE tensor_add vs broadcast tile,
                                            the only PSUM->SBUF pass)
      one 0.5 MiB bf16 HWDGE store covering both batches -> out[b0/b1,n,c]

Numerics: theta, W, x, h rounded to bf16 => ~5e-3 rel err (gate is 2e-2).
"""

import os
import sys

sys.path.insert(0, "/opt/trn_rl_repo")

import numpy as np

import concourse.bacc as bacc
import concourse.mybir as mybir
from concourse import tile
from concourse.bass_utils import run_bass_kernel_spmd

dt = mybir.dt
AF = mybir.ActivationFunctionType

B, C, H, W = 32, 64, 128, 128
HW = H * W
NCORES = 8
BL = B // NCORES  # batches per core
PAIRS = BL // 2

DMACHUNK = 4096  # cols per chunk (2 MiB f32 load, 1 MiB bf16 store)
SUB = 1024  # cols per phase tile (2 PSUM banks); matmuls are 512/128-col

VARIANT = os.environ.get("BASS_VARIANT", "v2")


def _build(variant: str):
    nc = bacc.Bacc("TRN2", target_bir_lowering=False, debug=False)

    x_d = nc.dram_tensor("x", [BL, C, HW], dt.float32, kind="ExternalInput")
    th_d = nc.dram_tensor("theta", [C, C], dt.float32, kind="ExternalInput")
    wl_d = nc.dram_tensor("W_lin", [C, C], dt.float32, kind="ExternalInput")
    bl_d = nc.dram_tensor("b_lin", [C], dt.float32, kind="ExternalInput")
    out_d = nc.dram_tensor("out", [BL, HW, C], dt.bfloat16, kind="ExternalOutput")

    R = DMACHUNK // 128  # output rows per partition per chunk (16)
    WW = SUB // R  # w-window per sub in the relu scatter (32)
    NSUB = DMACHUNK // SUB  # subs per chunk (4)
    NCHUNK = HW // DMACHUNK  # chunks per pair (8)

    with tile.TileContext(nc) as tc:
        with (
            tc.tile_pool(name="const", bufs=1) as const,
            tc.tile_pool(name="xfp", bufs=1) as xfp,
            tc.tile_pool(name="xp", bufs=6) as xp,
            tc.tile_pool(name="hp", bufs=3) as hp,
            tc.tile_pool(name="op", bufs=4) as op_,
        ):
            # ---------- first x loads: issue before anything else ----------
            # sync ring takes even chunks, scalar ring odd chunks. Tiles for
            # the whole kernel come from xp (bufs=4), so the first 4 loads can
            # be in flight while the constants pipeline below resolves.
            xsrc = x_d[:].rearrange("b c n -> (b c) n")
            xf_tiles = {}
            x_tiles = {}

            def load_chunk(g):
                pair, ci = divmod(g, NCHUNK)
                b0, n0 = 2 * pair, ci * DMACHUNK
                s = xsrc[b0 * C : (b0 + 2) * C, n0 : n0 + DMACHUNK]
                if g >= 1:
                    # SWDGE cast-load: bf16 lands directly -- half the SBUF
                    # fabric bytes, no compute cost
                    x2 = xp.tile([128, DMACHUNK], dt.bfloat16, tag="x2")
                    nc.gpsimd.dma_start(x2[:], s)
                    x_tiles[g] = x2
                else:
                    # chunk 0 rides scalar-HWDGE f32 (the SWDGE path sits
                    # behind the engine preamble); bf16 cast split ACT/DVE
                    xf = xfp.tile([128, DMACHUNK], dt.float32, tag="xf")
                    nc.scalar.dma_start(xf[:], s)
                    xf_tiles[g] = xf

            def cast_chunk(g):
                if g not in xf_tiles:
                    return
                xf = xf_tiles.pop(g)
                x2 = xp.tile([128, DMACHUNK], dt.bfloat16, tag="x2")
                q = DMACHUNK // 4
                for i in range(4):
                    eng = nc.scalar if i % 2 == 0 else nc.vector
                    sl = slice(i * q, (i + 1) * q)
                    if i % 2 == 0:
                        nc.scalar.activation(x2[:, sl], xf[:, sl], AF.Copy)
                    else:
                        nc.vector.tensor_copy(x2[:, sl], xf[:, sl])
                x_tiles[g] = x2

            load_chunk(0)

            # ---------------- constants (no PE transposes, no identity) ----
            # blockdiag targets zeroed on gpsimd; theta/W/bias DMAs on the
            # sync HWDGE ring (tiny; stores only start much later); DVE does
            # the 32x32 block transposes and the quad copies.
            ablk = const.tile([128, 128], dt.bfloat16, tag="ablk")
            wblk = const.tile([128, 128], dt.bfloat16, tag="wblk")
            ones_r = const.tile([1, 128], dt.float32, tag="ones_r")
            load_chunk(1)
            nc.gpsimd.memset(ablk[:], 0.0)
            nc.gpsimd.memset(wblk[:], 0.0)
            nc.gpsimd.memset(ones_r[:], 1.0)

            th_t = const.tile([64, 64], dt.float32, tag="th_t")
            wl_t = const.tile([64, 64], dt.float32, tag="wl_t")
            bias_r = const.tile([1, 64], dt.float32, tag="bias_r")
            nc.sync.dma_start(th_t[:], th_d[:])
            nc.sync.dma_start(wl_t[:], wl_d[:])
            nc.sync.dma_start(bias_r[:], bl_d[:].rearrange("(one c) -> one c", one=1))
            load_chunk(2)
            load_chunk(3)
            load_chunk(4)
            cast_chunk(0)

            # full 64x64 transposes out of 32x32 DVE block transposes
            thT = const.tile([64, 64], dt.float32, tag="thT")
            wlT = const.tile([64, 64], dt.float32, tag="wlT")
            for src_t, dst_t in ((th_t, thT), (wl_t, wlT)):
                for i in (0, 32):
                    for j in (0, 32):
                        nc.vector.transpose(
                            dst_t[i : i + 32, j : j + 32],
                            src_t[j : j + 32, i : i + 32],
                        )
            # Ablk = -0.5 * blockdiag(theta^T, theta^T)  (lhsT of mm1, bf16)
            nc.vector.tensor_scalar_mul(ablk[0:64, 0:64], thT[:], -0.5)
            nc.vector.tensor_scalar_mul(ablk[64:128, 64:128], thT[:], -0.5)
            # Wblk = blockdiag(W_lin^T, W_lin^T)  (rhs of mm2, bf16)
            nc.vector.tensor_copy(wblk[0:64, 0:64], wlT[:])
            nc.vector.tensor_copy(wblk[64:128, 64:128], wlT[:])

            # bias_tile [128, (t4 bi c)]: b_lin broadcast to all partitions
            # via a K=1 PE matmul, then log-doubled across the free dim
            psc_cm = tc.tile_pool(name="psc", bufs=1, space="PSUM")
            psc = psc_cm.__enter__()
            psb = psc.tile([128, 64], dt.float32, tag="psb")
            nc.tensor.matmul(psb[:], ones_r[:], bias_r[:], start=True, stop=True)
            bias_t = const.tile([128, 1024], dt.float32, tag="bias_t")
            nc.vector.tensor_copy(bias_t[:, 0:64], psb[:])
            nc.vector.tensor_copy(bias_t[:, 64:128], bias_t[:, 0:64])
            nc.vector.tensor_copy(bias_t[:, 128:256], bias_t[:, 0:128])
            nc.vector.tensor_copy(bias_t[:, 256:512], bias_t[:, 0:256])
            nc.vector.tensor_copy(bias_t[:, 512:1024], bias_t[:, 0:512])
            bias_v = bias_t[:].rearrange("p (bi t c) -> p bi t c", bi=2, c=64)

            psc_cm.__exit__(None, None, None)
            ps1p_cm = tc.tile_pool(name="ps1p", bufs=2, space="PSUM")
            ps2p_cm = tc.tile_pool(name="ps2p", bufs=2, space="PSUM")
            ps1p = ps1p_cm.__enter__()
            ps2p = ps2p_cm.__enter__()

            # ---------------- main loop (one-chunk software pipeline) ----
            # Phase 1 of chunk g (mm1 + relu) is emitted BEFORE phase 2 of
            # chunk g-1 (mm2' + bias-add + store), so the PE queue runs
            # mm1(g) ahead of mm2'(g-1) and ACT's relu stream never waits a
            # whole mm2' group.
            loaded = set(range(5))

            def phase1(g):
                if g + 5 < PAIRS * NCHUNK and (g + 5) not in loaded:
                    loaded.add(g + 5)
                    load_chunk(g + 5)
                x2 = x_tiles.pop(g)
                h = hp.tile([128, DMACHUNK], dt.bfloat16, tag="h")
                hv = h[:].rearrange("p (t q) -> p t q", t=R)
                for s in range(NSUB):
                    ps1 = ps1p.tile([128, SUB], dt.float32, tag="ps1")
                    for m in range(SUB // 512):
                        c0 = s * SUB + m * 512
                        nc.tensor.matmul(
                            ps1[:, m * 512 : (m + 1) * 512],
                            ablk[:],
                            x2[:, c0 : c0 + 512],
                            start=True,
                            stop=True,
                        )
                    # relu + permuted scatter over the whole 1024-col tile
                    ps1v = ps1[:].rearrange("p (a r) -> p r a", r=R)
                    nc.scalar.activation(
                        hv[:, :, WW * s : WW * (s + 1)], ps1v, AF.Relu
                    )
                # cast the chunk-after-next now, between this chunk's relus
                # and the next chunk's, so ACT never stalls ahead of a relu
                if g + 2 < PAIRS * NCHUNK:
                    cast_chunk(g + 2)
                return h

            def phase2(g, h):
                pair, ci = divmod(g, NCHUNK)
                b0, n0 = 2 * pair, ci * DMACHUNK
                o = op_.tile([128, DMACHUNK], dt.bfloat16, tag="o")
                obv = o[:].rearrange("p (bi tc c) -> p bi tc c", bi=2, c=64)
                TG = SUB // 128  # mm2' blocks per group (8)
                for s in range(NSUB):
                    ps2 = ps2p.tile([128, SUB], dt.float32, tag="ps2")
                    for t in range(TG):
                        blk = (TG * s + t) * 128
                        nc.tensor.matmul(
                            ps2[:, t * 128 : (t + 1) * 128],
                            h[:, blk : blk + 128],
                            wblk[:],
                            start=True,
                            stop=True,
                        )
                    p2v = ps2[:].rearrange("p (t bi c) -> p bi t c", t=TG, c=64)
                    dst = obv[:, :, TG * s : TG * (s + 1), :]
                    nc.vector.tensor_add(dst, p2v, bias_v)
                dd = out_d[b0 : b0 + 2, n0 : n0 + DMACHUNK, :].rearrange(
                    "bi (p tc) c -> p bi tc c", p=128
                )
                nc.sync.dma_start(dd, obv)

            prev = None
            for g in range(PAIRS * NCHUNK):
                h = phase1(g)
                if prev is not None:
                    phase2(prev[0], prev[1])
                prev = (g, h)
            phase2(prev[0], prev[1])
            ps2p_cm.__exit__(None, None, None)
            ps1p_cm.__exit__(None, None, None)

    nc.compile()
    return nc


def _ensure_ntff_hook():
    """Register the axon NTFF profile hook (profiling only; best-effort).

    The agent image's ``antenv`` lacks ``axon_hooks``, so ``trace=True`` in
    ``run_bass_kernel_spmd`` would ImportError. Recreate the module with the
    same ctypes hook ``trn_agent_boot.trn_boot`` would have registered.
    """
    import contextlib
    import ctypes
    import types

    if "antenv.axon_hooks" in sys.modules:
        return
    so_path = "/opt/axon/libaxon_pjrt.so"
    try:
        lib = ctypes.CDLL(so_path)
        lib.axon_start_nrt_profile.argtypes = [
            ctypes.POINTER(ctypes.c_int64),
            ctypes.c_size_t,
        ]
        lib.axon_start_nrt_profile.restype = ctypes.c_int64
        lib.axon_stop_nrt_profile.argtypes = [ctypes.c_char_p]
        lib.axon_stop_nrt_profile.restype = ctypes.c_int64
    except (OSError, AttributeError):
        lib = None

    @contextlib.contextmanager
    def _hook(output_dir, device_ids):
        import jax

        jax.devices()
        if device_ids:
            ids = (ctypes.c_int64 * len(device_ids))(*device_ids)
            rc = lib.axon_start_nrt_profile(ids, len(device_ids))
        else:
            rc = lib.axon_start_nrt_profile(None, 0)
        if rc != 0:
            raise RuntimeError(f"axon_start_nrt_profile rc={rc}")
        try:
            yield
        finally:
            n = lib.axon_stop_nrt_profile(str(output_dir).encode())
            print(f"ntff profile: {n} file(s) written to {output_dir}")

    hook = _hook if lib is not None else None
    mod = types.ModuleType("antenv.axon_hooks")
    mod.get_axon_ntff_profile_hook = lambda: hook
    mod.set_axon_ntff_profile_hook = lambda h: None
    sys.modules["antenv.axon_hooks"] = mod


_NC_CACHE = {}


def _get_nc(variant: str):
    if variant not in _NC_CACHE:
        _NC_CACHE[variant] = _build(variant)
    return _NC_CACHE[variant]


def _run(inputs: dict, trace: bool = False, variant: str | None = None):
    variant = variant or VARIANT
    if trace:
        _ensure_ntff_hook()
    nc = _get_nc(variant)
    x = np.ascontiguousarray(inputs["x"], dtype=np.float32)
    theta = np.ascontiguousarray(inputs["theta"], dtype=np.float32)
    w_lin = np.ascontiguousarray(inputs["W_lin"], dtype=np.float32)
    b_lin = np.ascontiguousarray(inputs["b_lin"], dtype=np.float32)
    in_maps = [
        {
            "x": np.ascontiguousarray(x[i * BL : (i + 1) * BL].reshape(BL, C, HW)),
            "theta": theta,
            "W_lin": w_lin,
            "b_lin": b_lin,
        }
        for i in range(NCORES)
    ]
    # Occasionally the first execution of a freshly-loaded NEFF fails with
    # NRT_EXEC_UNIT_UNRECOVERABLE; a retry on the recovered device succeeds.
    import time

    last_err = None
    for attempt in range(4):
        try:
            res = run_bass_kernel_spmd(
                nc,
                in_maps,
                core_ids=list(range(NCORES)),
                trace=trace and attempt == 0,
            )
            break
        except Exception as e:  # noqa: BLE001
            last_err = e
            try:  # drop the (possibly dead) PJRT client; next call re-inits
                import jax

                jax.clear_caches()
                jax.extend.backend.clear_backends()
            except Exception:  # noqa: BLE001
                pass
            time.sleep(10 * (attempt + 1))
    else:
        raise last_err
    shards = [
        np.asarray(r["out"]).astype(np.float32).reshape(BL, C, H, W)
        for r in res.results
    ]
    return np.concatenate(shards, axis=0), res


def kernel(x, theta, W_lin, b_lin):
    out, _ = _run({"x": x, "theta": theta, "W_lin": W_lin, "b_lin": b_lin})
    return out
